# revision 33
# baseline (speedup 1.0000x reference)
"""nn_AttnDecoderCell — Trainium2 Bass kernel (8 NeuronCores).

kernel(**inputs) takes the FULL unsharded f32 inputs and returns the full
s_t [512, 1024] float32.

Sharding: 2D. The attention phase is data-parallel over batch (64 rows per
core — constants is the dominant HBM traffic and scales inversely with the
batch shard). The GRU phase is column-parallel (128 output columns per
core) so each core loads only 1/8 of the GRU weights; the activations are
exchanged on-chip with collectives (v: AllGather for the U-matmul operand +
AllToAll for each core's column slice; r: AllGather of column shards).
All inputs are host-cast to bf16 (halves HBM traffic; PE at 1 cycle/row).

Per-core program:
 - Attention: C streamed as [l(128p), d] bf16 tiles; energy = C·w_att
   fused multiply+reduce on DVE (Pool-mult + ACT-reduce for 4/8 of tiles);
   softmax normalization folded into exp columns per group (sumexp via
   ones-matmul, reciprocal, partition-broadcast, in-place scale) so v comes
   out of PSUM already normalized; v rows accumulated with exp column
   [128,1] stationary and C [128,512] moving, 4 batches per PSUM bank via
   col-group tile_position; one ACT copy stages 4 rows, two small DMAs
   scatter them to the collective bounce buffers.
 - GRU (transposed, column-sharded): all operands arrive k-major via
   DMA-transpose loads (state.T, x.T, v.T) or are produced transposed
   (r.T via AllGather of per-core r.T shards; (state*r).T = state.T ⊙ r.T
   elementwise on DVE — transposes commute with elementwise ops). Matmuls
   put the 128 output columns on PSUM partitions and batch 512 on the
   moving dim; per-gate bias is a per-partition vector folded into the
   ACT sigmoid/tanh. Final combine + output stay transposed; the host
   transposes back (free) and concatenates core outputs on axis 1.
"""

from contextlib import ExitStack

import numpy as np

import concourse.bacc as bacc
import concourse.bass as bass
import concourse.tile as tile
import concourse.mybir as mybir
from concourse.bass_utils import run_bass_kernel_spmd

f32 = mybir.dt.float32
bf16 = mybir.dt.bfloat16
BF16_NP = mybir.dt.np(bf16)
AF = mybir.ActivationFunctionType
ALU = mybir.AluOpType

B, L, D, DIN = 512, 256, 1024, 1024
N_CORES = 8
Bc = B // N_CORES          # 64 attention batch rows per core
DC = D // N_CORES          # 128 GRU output columns per core
LT = L // 128              # 2 l-tiles
KW = D // 128              # 8 k-tiles for W matmuls
KU = (DIN + D) // 128      # 16 k-tiles for U matmuls
G = 4                      # batches per attention group (psum col-groups)
NG = Bc // G               # 16 groups


def _build(loop_n=1, c_bufs=3):
    nc = bacc.Bacc("TRN2", target_bir_lowering=False, debug=False,
                   num_devices=N_CORES)
    x_d = nc.dram_tensor("x", [B, DIN], bf16, kind="ExternalInput").ap()
    s_d = nc.dram_tensor("state", [B, D], bf16, kind="ExternalInput").ap()
    sct_d = nc.dram_tensor("state_colsT", [DC, B], bf16,
                           kind="ExternalInput").ap()
    c_d = nc.dram_tensor("constants", [Bc, L, D], bf16,
                         kind="ExternalInput").ap()
    watt_d = nc.dram_tensor("w_att_c", [1, D], bf16, kind="ExternalInput").ap()
    w_g, u_g, b_g = {}, {}, {}
    for g in "zrh":
        w_g[g] = nc.dram_tensor(f"w_{g}", [128, KW, DC], bf16,
                                kind="ExternalInput").ap()
        u_g[g] = nc.dram_tensor(f"u_{g}", [128, KU, DC], bf16,
                                kind="ExternalInput").ap()
        b_g[g] = nc.dram_tensor(f"b_{g}", [DC, 1], bf16,
                                kind="ExternalInput").ap()
    vsel_d = nc.dram_tensor("vsel", [128, KW, DC], bf16,
                            kind="ExternalInput").ap()
    o_d = nc.dram_tensor("out", [DC, B], f32, kind="ExternalOutput").ap()

    with tile.TileContext(nc) as tc:
      def body(_i):
        es = ExitStack()
        small = es.enter_context(tc.tile_pool(name="small", bufs=1))
        cpool = es.enter_context(tc.tile_pool(name="cpool", bufs=c_bufs))
        scr = es.enter_context(tc.tile_pool(name="scr", bufs=1))
        st4 = es.enter_context(tc.tile_pool(name="st4", bufs=2))
        dram = es.enter_context(tc.tile_pool(name="dram", bufs=1,
                                             space="DRAM"))
        psV = es.enter_context(tc.tile_pool(name="psV", bufs=1, space="PSUM"))
        psG = es.enter_context(tc.tile_pool(name="psG", bufs=3, space="PSUM"))
        psA = es.enter_context(tc.tile_pool(name="psA", bufs=2, space="PSUM"))

        # ---------------- setup ----------------
        wc_rep = small.tile([128, D], bf16)
        nc.sync.dma_start(
            wc_rep[:],
            bass.AP(tensor=watt_d.tensor, offset=0, ap=[[0, 128], [1, D]]))
        ones_col = small.tile([128, 1], bf16)
        nc.vector.memset(ones_col[:], 1.0)
        sct_sb = small.tile([DC, B], bf16)
        nc.sync.dma_start(sct_sb[:], sct_d[:])
        bT = {}
        for g in "zrh":
            bT[g] = small.tile([DC, 1], bf16, name=f"bT_{g}")
            nc.sync.dma_start(bT[g][:], b_g[g][:])
        wts, uts = {}, {}
        for g in "zrh":
            wts[g] = small.tile([128, KW, DC], bf16, name=f"wt_{g}")
            nc.sync.dma_start(wts[g][:], w_g[g][:])
            uts[g] = small.tile([128, KU, DC], bf16, name=f"ut_{g}")
            nc.sync.dma_start(uts[g][:], u_g[g][:])
        vsel = small.tile([128, KW, DC], bf16)
        nc.sync.dma_start(vsel[:], vsel_d[:])

        sT = small.tile([128, KW, B], bf16)
        xT = small.tile([128, KW, B], bf16)
        for ch in range(KW):
            nc.sync.dma_start_transpose(sT[:, ch, :],
                                        s_d[:, ch * 128:(ch + 1) * 128])
            nc.sync.dma_start_transpose(xT[:, ch, :],
                                        x_d[:, ch * 128:(ch + 1) * 128])

        # collective bounce buffers (HBM)
        v_b = dram.tile([Bc, D], bf16)
        v_ag = dram.tile([N_CORES, Bc, D], bf16, addr_space="Shared")
        r_b = dram.tile([DC, B], bf16)
        r_ag = dram.tile([N_CORES, DC, B], bf16, addr_space="Shared")

        # ---------------- attention ----------------
        eT = small.tile([128, LT * Bc], f32)
        expT = small.tile([128, LT * Bc], bf16)
        vp = psV.tile([128, 2 * 512], f32)
        nc.vector.memset(vp[:], 0.0)

        for gi in range(NG):
            b0 = gi * G
            ct = cpool.tile([128, G, LT, D], bf16, tag="ct")
            nc.sync.dma_start(
                ct[:],
                c_d[b0:b0 + G].rearrange("b (t p) d -> p b t d", p=128))
            # energy: e[l, b] = sum_d C[l, d] * w_att[d]
            for bi in range(G):
                for lt in range(LT):
                    prod = scr.tile([128, D], bf16, tag=f"prod{bi}_{lt}")
                    col = lt * Bc + b0 + bi
                    if (bi * LT + lt) % 2 == 1:
                        nc.gpsimd.tensor_tensor(
                            out=prod[:], in0=ct[:, bi, lt, :],
                            in1=wc_rep[:], op=ALU.mult)
                        nc.scalar.activation(
                            out=prod[:], in_=prod[:], func=AF.Copy,
                            accum_out=eT[:, col:col + 1])
                    else:
                        nc.vector.scalar_tensor_tensor(
                            out=prod[:], in0=ct[:, bi, lt, :], scalar=1.0,
                            in1=wc_rep[:], op0=ALU.bypass, op1=ALU.mult,
                            accum_out=eT[:, col:col + 1])
            for lt in range(LT):
                col = lt * Bc + b0
                nc.scalar.activation(out=expT[:, col:col + G],
                                     in_=eT[:, col:col + G], func=AF.Exp)
            # normalize the exp columns in place (softmax denominator)
            s4 = psA.tile([1, G], f32, name="s4", tag="s4")
            for lt in range(LT):
                nc.tensor.matmul(s4[:], ones_col[:],
                                 expT[:, lt * Bc + b0:lt * Bc + b0 + G],
                                 start=(lt == 0), stop=(lt == LT - 1),
                                 skip_group_check=True)
            rec4 = small.tile([1, G], f32, name="rec4", tag="rec4", bufs=2)
            nc.vector.reciprocal(rec4[:], s4[:])
            rec4_rep = small.tile([128, G], f32, name="rec4_rep",
                                  tag="rec4_rep", bufs=2)
            nc.gpsimd.partition_broadcast(rec4_rep[:], rec4[:])
            for lt in range(LT):
                col = lt * Bc + b0
                nc.vector.tensor_tensor(out=expT[:, col:col + G],
                                        in0=expT[:, col:col + G],
                                        in1=rec4_rep[:], op=ALU.mult)
            # v rows into psum partitions {0,32,64,96} via col-groups
            for lt in range(LT):
                for chn in range(2):
                    for j in range(G):
                        nc.tensor.matmul(
                            vp[32 * j:32 * j + 1, chn * 512:(chn + 1) * 512],
                            expT[:, lt * Bc + b0 + j:lt * Bc + b0 + j + 1],
                            ct[:, j, lt, chn * 512:(chn + 1) * 512],
                            start=(lt == 0), stop=(lt == LT - 1),
                            skip_group_check=True, tile_position=(0, 32 * j))
            stage4 = st4.tile([128, D], bf16, tag="stage4")
            nc.scalar.copy(stage4[:], vp[:])
            nc.sync.dma_start(v_b[b0:b0 + G, :], stage4[0:97:32, :])

        # ---------------- GRU (transposed, col-sharded) ----------------
        grp = [list(range(N_CORES))]
        gp_of = {}

        def gateT_mms(g, ks, movs, start=False, stop=False):
            if g not in gp_of:
                gp_of[g] = psG.tile([DC, B], f32, name=f"gp_{g}", tag="gp")
            gp = gp_of[g]
            for i, (k, mov) in enumerate(zip(ks, movs)):
                wt = wts[g] if k < KW else uts[g]
                kk = k if k < KW else k - KW
                nc.tensor.matmul(gp[:], wt[:, kk, :], mov,
                                 start=start and i == 0,
                                 stop=stop and i == len(ks) - 1,
                                 skip_group_check=True)

        def w_part(g, lhsW3, **kw):
            gateT_mms(g, range(KW), [lhsW3[:, k, :] for k in range(KW)], **kw)

        def ux_part(g, **kw):
            gateT_mms(g, range(KW, 2 * KW),
                      [xT[:, k, :] for k in range(KW)], **kw)

        def uv_part(g, vT, **kw):
            gateT_mms(g, range(2 * KW, 3 * KW),
                      [vT[:, k, :] for k in range(KW)], **kw)

        def gate_act(g, out_sb, func):
            nc.scalar.activation(out=out_sb[:], in_=gp_of[g][:], func=func,
                                 bias=bT[g][:])

        zT_sb = small.tile([DC, B], bf16)
        rT_sb = small.tile([DC, B], bf16)
        hT_sb = small.tile([DC, B], bf16)

        # W and Ux parts of z/r don't need v — they overlap the v AllGather
        w_part("z", sT, start=True)
        ux_part("z")
        w_part("r", sT, start=True)
        ux_part("r")

        nc.gpsimd.collective_compute(
            "AllGather", ALU.bypass, replica_groups=grp,
            ins=[v_b.opt()], outs=[v_ag.opt()])
        vT = small.tile([128, KW, B], bf16)
        v_full = v_ag.rearrange("c b d -> (c b) d")
        for ch in range(KW):
            nc.sync.dma_start_transpose(vT[:, ch, :],
                                        v_full[:, ch * 128:(ch + 1) * 128])

        uv_part("z", vT, stop=True)
        gate_act("z", zT_sb, AF.Sigmoid)
        uv_part("r", vT, stop=True)
        gate_act("r", rT_sb, AF.Sigmoid)
        nc.sync.dma_start(r_b[:], rT_sb[:])
        nc.gpsimd.collective_compute(
            "AllGather", ALU.bypass, replica_groups=grp,
            ins=[r_b.opt()], outs=[r_ag.opt()])

        # h's Ux/Uv parts and the v column-selection overlap the r AllGather
        ux_part("h", start=True)
        uv_part("h", vT)
        oV = psA.tile([DC, B], f32, name="oV", tag="oV", bufs=1)
        for ch in range(KW):
            nc.tensor.matmul(oV[:], vsel[:, ch, :], vT[:, ch, :],
                             start=(ch == 0), stop=(ch == KW - 1),
                             skip_group_check=True)

        rT_full = small.tile([128, KW, B], bf16)
        nc.sync.dma_start(rT_full[:], r_ag.rearrange("ch dc b -> dc ch b"))
        rsT = small.tile([128, KW, B], bf16)
        nc.vector.tensor_tensor(out=rsT[:], in0=sT[:], in1=rT_full[:],
                                op=ALU.mult)
        w_part("h", rsT, stop=True)
        gate_act("h", hT_sb, AF.Tanh)

        d1 = small.tile([DC, B], bf16)
        nc.vector.tensor_tensor(out=d1[:], in0=sct_sb[:], in1=hT_sb[:],
                                op=ALU.subtract)
        d2 = small.tile([DC, B], bf16)
        nc.gpsimd.tensor_tensor(out=d2[:], in0=d1[:], in1=zT_sb[:],
                                op=ALU.mult)
        d3 = small.tile([DC, B], bf16)
        nc.vector.tensor_tensor(out=d3[:], in0=d2[:], in1=hT_sb[:],
                                op=ALU.add)
        oT = small.tile([DC, B], f32)
        nc.vector.tensor_tensor(out=oT[:], in0=d3[:], in1=oV[:],
                                op=ALU.add)
        nc.sync.dma_start(o_d[:], oT[:])
        es.close()

      # collectives cannot be replayed inside a For_i hardware loop, so
      # multi-iteration (timing) builds unroll the body instead
      for i in range(loop_n):
          body(i)

    nc.compile()
    return nc


_NC_CACHE = {}


def _get_nc(loop_n=1):
    if loop_n not in _NC_CACHE:
        _NC_CACHE[loop_n] = _build(loop_n=loop_n)
    return _NC_CACHE[loop_n]


def _in_maps(inputs):
    def bf(a):
        return np.ascontiguousarray(np.asarray(a).astype(BF16_NP))

    def pack(w):
        # [T*128, DC] -> [128, T, DC] partition-major k-tile layout
        t = w.shape[0] // 128
        return np.ascontiguousarray(
            w.reshape(t, 128, w.shape[1]).transpose(1, 0, 2))

    watt_c = bf(np.asarray(inputs["w_att"], np.float32)[D:2 * D, 0]
                ).reshape(1, D)
    x_bf = bf(inputs["x"])
    s_bf = bf(inputs["state"])
    c_bf = bf(inputs["constants"])
    w_bf = {g: bf(inputs[f"w_{g}"]) for g in "zrh"}
    u_bf = {g: bf(inputs[f"u_{g}"]) for g in "zrh"}
    b_bf = {g: bf(inputs[f"b_{g}"]) for g in "zrh"}
    maps = []
    for c in range(N_CORES):
        lo, hi = c * Bc, (c + 1) * Bc
        cl, ch = c * DC, (c + 1) * DC
        m = {
            "x": x_bf,
            "state": s_bf,
            "state_colsT": np.ascontiguousarray(s_bf[:, cl:ch].T),
            "constants": np.ascontiguousarray(c_bf[lo:hi]),
            "w_att_c": watt_c,
        }
        for g in "zrh":
            m[f"w_{g}"] = pack(w_bf[g][:, cl:ch])
            m[f"u_{g}"] = pack(u_bf[g][:, cl:ch])
            m[f"b_{g}"] = np.ascontiguousarray(b_bf[g][cl:ch].reshape(DC, 1))
        vsel = np.zeros((128, KW, DC), dtype=BF16_NP)
        vsel[:, c, :] = np.eye(128, dtype=BF16_NP)
        m["vsel"] = vsel
        maps.append(m)
    return maps


def kernel(**inputs) -> np.ndarray:
    nc = _get_nc(loop_n=1)
    res = run_bass_kernel_spmd(nc, _in_maps(inputs),
                               core_ids=list(range(N_CORES)))
    return np.concatenate(
        [res.results[c]["out"].T for c in range(N_CORES)],
        axis=1).astype(np.float32)


# revision 38
# speedup vs baseline: 2.0305x; 2.0305x over previous
"""nn_AttnDecoderCell — Trainium2 Bass kernel (8 NeuronCores, data-parallel).

kernel(**inputs) takes the FULL unsharded f32 inputs (x[512,1024],
state[512,1024], constants[512,256,1024], w_att[2048,1], b_att[1],
w_z/u_z/b_z, w_r/u_r/b_r, w_h/u_h/b_h) and returns the full s_t
[512, 1024] float32.

Sharding: batch dim split 64 rows per core; weights replicated. All inputs
are cast to bf16 on the host before upload — halves HBM traffic (the kernel
is DMA-bound) and runs the PE at 1 cycle/row instead of f32's 4.

Per-core program:
 - Attention: C streamed as [l(128p), d] bf16 tiles; energy = fused
   multiply+free-dim-reduce (scalar_tensor_tensor with accum_out) against a
   broadcast w_att row, split DVE/Pool (the state@w_att and b_att terms are
   softmax-shift-invariant); exp on ACT; v rows accumulated in PSUM with the
   exp column [128,1] stationary and C [128,512] moving, 4 batches landing
   on psum partitions {0,32,64,96} via col-group tile_position so one ACT
   copy stages 4 rows at once; staged rows PE-transposed into vT[128,8,64]
   (the layout the GRU needs) and transposed back once at the end for the
   [64,1024] row-layout v; sumexp via 2 ones-matmuls; normalization applied
   to vT once after the loop.
 - GRU: bf16 matmuls with batch on PSUM partitions; state.T/x.T/(r*state).T
   built via PE transposes; weights streamed from DRAM bf16 one whole matrix
   per DMA; bias added with a rank-1 ones matmul into the same PSUM group;
   sigmoid/tanh on ACT straight from PSUM; final combine on DVE/Pool.
"""

from contextlib import ExitStack

import numpy as np

import concourse.bacc as bacc
import concourse.bass as bass
import concourse.tile as tile
import concourse.mybir as mybir
from concourse.bass_utils import run_bass_kernel_spmd
from concourse.masks import make_identity

f32 = mybir.dt.float32
bf16 = mybir.dt.bfloat16
BF16_NP = mybir.dt.np(bf16)
AF = mybir.ActivationFunctionType
ALU = mybir.AluOpType

B, L, D, DIN = 512, 256, 1024, 1024
N_CORES = 8
Bc = B // N_CORES          # 64 batch rows per core
LT = L // 128              # 2 l-tiles
KW = D // 128              # 8 k-tiles for W matmuls
KU = (DIN + D) // 128      # 16 k-tiles for U matmuls
G = 4                      # batches per attention group (psum col-groups)
NG = Bc // G               # 16 groups


def _build(loop_n=1, c_bufs=3):
    nc = bacc.Bacc("TRN2", target_bir_lowering=False, debug=False,
                   num_devices=N_CORES)
    x_d = nc.dram_tensor("x", [Bc, DIN], bf16, kind="ExternalInput").ap()
    s_d = nc.dram_tensor("state", [Bc, D], bf16, kind="ExternalInput").ap()
    c_d = nc.dram_tensor("constants", [Bc, L, D], bf16,
                         kind="ExternalInput").ap()
    watt_d = nc.dram_tensor("w_att_c", [1, D], bf16, kind="ExternalInput").ap()
    w_g, u_g, b_g = {}, {}, {}
    for g in "zrh":
        # host-packed partition-major k-tile layouts (contiguous DMA blobs)
        w_g[g] = nc.dram_tensor(f"w_{g}", [128, KW, D], bf16,
                                kind="ExternalInput").ap()
        u_g[g] = nc.dram_tensor(f"u_{g}", [128, KU, D], bf16,
                                kind="ExternalInput").ap()
        b_g[g] = nc.dram_tensor(f"b_{g}", [1, D], bf16,
                                kind="ExternalInput").ap()
    o_d = nc.dram_tensor("out", [Bc, D], f32, kind="ExternalOutput").ap()

    with tile.TileContext(nc) as tc:
      def body(_i):
        es = ExitStack()
        small = es.enter_context(tc.tile_pool(name="small", bufs=1))
        cpool = es.enter_context(tc.tile_pool(name="cpool", bufs=c_bufs))
        scr = es.enter_context(tc.tile_pool(name="scr", bufs=2))
        st4 = es.enter_context(tc.tile_pool(name="st4", bufs=2))
        wpool = es.enter_context(tc.tile_pool(name="wpool", bufs=2))
        upool = es.enter_context(tc.tile_pool(name="upool", bufs=2))
        psV = es.enter_context(tc.tile_pool(name="psV", bufs=1, space="PSUM"))
        psT = es.enter_context(tc.tile_pool(name="psT", bufs=2, space="PSUM"))
        psG = es.enter_context(tc.tile_pool(name="psG", bufs=2, space="PSUM"))

        # ---------------- setup ----------------
        ident = small.tile([128, 128], bf16)
        make_identity(nc, ident[:])
        wc_rep = small.tile([128, D], bf16)
        nc.sync.dma_start(
            wc_rep[:],
            bass.AP(tensor=watt_d.tensor, offset=0, ap=[[0, 128], [1, D]]))
        ones_col = small.tile([128, 1], bf16)
        nc.vector.memset(ones_col[:], 1.0)
        ones_row = small.tile([1, Bc], bf16)
        nc.vector.memset(ones_row[:], 1.0)
        brow = {}
        for g in "zrh":
            brow[g] = small.tile([1, D], bf16, name=f"brow_{g}")
            nc.sync.dma_start(brow[g][:], b_g[g][:])

        # weight loads are issued AFTER the C stream (DMA drains in FIFO
        # order; C feeds the attention phase which runs first); contiguous
        # host-packed blobs
        def load_w(g):
            wt = wpool.tile([128, KW, D], bf16, tag="wt", name=f"w{g}")
            nc.sync.dma_start(wt[:], w_g[g][:])
            ut = upool.tile([128, KU, D], bf16, tag="ut", name=f"u{g}")
            nc.sync.dma_start(ut[:], u_g[g][:])
            return wt, ut

        xs = small.tile([Bc, DIN], bf16)
        nc.sync.dma_start(xs[:], x_d[:])
        ss = small.tile([Bc, D], bf16)
        nc.sync.dma_start(ss[:], s_d[:])

        def transpose_to(dst3, src2d):
            n = dst3.shape[1]
            for ch in range(n):
                tp = psT.tile([128, 8, 128], bf16, name="tp", tag="tp")
                nc.tensor.transpose(tp[:, 0, :Bc],
                                    src2d[:, ch * 128:(ch + 1) * 128],
                                    ident[:Bc, :Bc])
                nc.vector.tensor_copy(out=dst3[:, ch, :], in_=tp[:, 0, :Bc])

        sT = small.tile([128, KW, Bc], bf16)
        transpose_to(sT, ss)
        xT = small.tile([128, KW, Bc], bf16)
        transpose_to(xT, xs)

        # ---------------- attention ----------------
        eT = small.tile([128, LT * Bc], f32)
        expT = small.tile([128, LT * Bc], bf16)
        vT = small.tile([128, KW, Bc], bf16)

        vp0 = psV.tile([128, 2 * 512], f32, name="vp0")
        nc.vector.memset(vp0[:], 0.0)
        vp1 = psV.tile([128, 2 * 512], f32, name="vp1")
        nc.vector.memset(vp1[:], 0.0)

        for gi in range(NG):
            vp = vp0 if gi % 2 == 0 else vp1
            b0 = gi * G
            ct = cpool.tile([128, G, LT, D], bf16, tag="ct")
            nc.sync.dma_start(
                ct[:],
                c_d[b0:b0 + G].rearrange("b (t p) d -> p b t d", p=128))
            # energy: e[l, b] = sum_d C[l, d] * w_att[d]
            # 5/8 fused on DVE; 3/8 as Pool-mult + ACT Copy-accum reduce
            for bi in range(G):
                for lt in range(LT):
                    prod = scr.tile([128, D], bf16, tag=f"prod{bi}_{lt}",
                                    bufs=1)
                    col = lt * Bc + b0 + bi
                    if (bi * LT + lt) % 2 == 1:
                        nc.gpsimd.tensor_tensor(
                            out=prod[:], in0=ct[:, bi, lt, :],
                            in1=wc_rep[:], op=ALU.mult)
                        nc.scalar.activation(
                            out=prod[:], in_=prod[:], func=AF.Copy,
                            accum_out=eT[:, col:col + 1])
                    else:
                        nc.vector.scalar_tensor_tensor(
                            out=prod[:], in0=ct[:, bi, lt, :], scalar=1.0,
                            in1=wc_rep[:], op0=ALU.bypass, op1=ALU.mult,
                            accum_out=eT[:, col:col + 1])
            for lt in range(LT):
                col = lt * Bc + b0
                nc.scalar.activation(out=expT[:, col:col + G],
                                     in_=eT[:, col:col + G], func=AF.Exp)
            # v rows into psum partitions {0,32,64,96} via col-groups
            for lt in range(LT):
                for chn in range(2):
                    for j in range(G):
                        nc.tensor.matmul(
                            vp[32 * j:32 * j + 1, chn * 512:(chn + 1) * 512],
                            expT[:, lt * Bc + b0 + j:lt * Bc + b0 + j + 1],
                            ct[:, j, lt, chn * 512:(chn + 1) * 512],
                            start=(lt == 0), stop=(lt == LT - 1),
                            skip_group_check=True, tile_position=(0, 32 * j))
            stage4 = st4.tile([128, D], bf16, tag="stage4")
            nc.scalar.copy(stage4[:], vp[:])
            # transpose the 4 staged rows into vT columns
            tp = psT.tile([128, 8, 128], bf16, name="tpv", tag="tp")
            for ch in range(KW):
                nc.tensor.transpose(tp[:, ch, :],
                                    stage4[:, ch * 128:(ch + 1) * 128],
                                    ident[:])
            nc.vector.tensor_copy(out=vT[:, :, b0:b0 + G],
                                  in_=tp[:, :, 0:97:32])

        # sumexp and normalization of vT
        s_ps = psT.tile([1, Bc], f32, name="s_ps", tag="tp")
        for lt in range(LT):
            nc.tensor.matmul(s_ps[:], ones_col[:],
                             expT[:, lt * Bc:(lt + 1) * Bc],
                             start=(lt == 0), stop=(lt == LT - 1),
                             skip_group_check=True)
        recip_row = small.tile([1, Bc], f32)
        nc.vector.reciprocal(recip_row[:], s_ps[:])
        recip_rep = small.tile([128, Bc], f32)
        nc.gpsimd.partition_broadcast(recip_rep[:], recip_row[:])
        for t in range(2):
            nc.vector.tensor_tensor(
                out=vT[:, 4 * t:4 * t + 4, :], in0=vT[:, 4 * t:4 * t + 4, :],
                in1=recip_rep[:, None, :].broadcast_to([128, 4, Bc]),
                op=ALU.mult)
        # transpose normalized vT back to row-layout v
        v_un = small.tile([Bc, D], bf16)
        for ch in range(KW):
            tb = psT.tile([128, 8, 128], bf16, name="tb", tag="tp")
            nc.tensor.transpose(tb[:Bc, 0, :], vT[:, ch, :], ident[:])
            nc.scalar.copy(v_un[:, ch * 128:(ch + 1) * 128], tb[:Bc, 0, :])

        # ---------------- GRU ----------------
        # Weight DMAs are stamped with tile_wait_until so the scheduler
        # cannot hoist them into the just-in-time C stream. uh/wh are
        # additionally slot-gated (3rd tile of their pools). Order matches
        # first use; wh last because h's W part has the shortest tail.
        with tc.tile_wait_until(0.088):
            wz, uz = load_w("z")
        with tc.tile_wait_until(0.105):
            wr, ur = load_w("r")
        with tc.tile_wait_until(0.118):
            uh = upool.tile([128, KU, D], bf16, tag="ut", name="uh")
            nc.sync.dma_start(uh[:], u_g["h"][:])
        with tc.tile_wait_until(0.140):
            wh = wpool.tile([128, KW, D], bf16, tag="wt", name="wh")
            nc.sync.dma_start(wh[:], w_g["h"][:])

        gp_of = {}

        def gate_mms(g, parts, wt=None, ut=None, lhsW3=None, stop=False):
            for chn in range(2):
                if (g, chn) not in gp_of:
                    gp_of[(g, chn)] = (
                        psG.tile([Bc, 512], f32, name=f"gp_{g}{chn}",
                                 tag="gp"), [True])
                gp, first = gp_of[(g, chn)]
                cs = slice(chn * 512, (chn + 1) * 512)

                def mm(lhs, rhs, last=False):
                    nc.tensor.matmul(gp[:], lhs, rhs, start=first[0],
                                     stop=stop and last,
                                     skip_group_check=True)
                    first[0] = False

                for p, part in enumerate(parts):
                    lastp = p == len(parts) - 1
                    if part == "w":
                        for k in range(KW):
                            mm(lhsW3[:, k, :], wt[:, k, cs],
                               last=lastp and k == KW - 1)
                    elif part == "ux":
                        for k in range(KW):
                            mm(xT[:, k, :], ut[:, k, cs],
                               last=lastp and k == KW - 1)
                    elif part == "uv":
                        for k in range(KW, KU):
                            mm(vT[:, k - KW, :], ut[:, k, cs],
                               last=lastp and k == KU - 1)
                    elif part == "bias":
                        mm(ones_row[:], brow[g][:, cs], last=lastp)

        def gate_act(g, out_sb, func):
            for chn in range(2):
                gp, _ = gp_of[(g, chn)]
                nc.scalar.activation(out=out_sb[:, chn * 512:(chn + 1) * 512],
                                     in_=gp[:], func=func)

        z_sb = small.tile([Bc, D], bf16)
        r_sb = small.tile([Bc, D], bf16)
        h_sb = small.tile([Bc, D], bf16)
        gate_mms("z", ["w", "ux", "uv", "bias"], wt=wz, ut=uz, lhsW3=sT,
                 stop=True)
        gate_act("z", z_sb, AF.Sigmoid)
        gate_mms("r", ["w", "ux", "uv", "bias"], wt=wr, ut=ur, lhsW3=sT,
                 stop=True)
        gate_act("r", r_sb, AF.Sigmoid)
        # h: Ux/Uv first (only need uh), W(rsT) last so the wh DMA and the
        # r -> rs -> rsT chain are both off the critical path as long as
        # possible
        gate_mms("h", ["ux", "uv", "bias"], ut=uh)
        rs_sb = small.tile([Bc, D], bf16)
        nc.vector.tensor_tensor(out=rs_sb[:], in0=ss[:], in1=r_sb[:],
                                op=ALU.mult)
        rsT = small.tile([128, KW, Bc], bf16)
        transpose_to(rsT, rs_sb)
        gate_mms("h", ["w"], wt=wh, lhsW3=rsT, stop=True)
        # per-half tail: tanh -> combine -> out DMA pipelines with the
        # second half's matmuls
        d1 = small.tile([Bc, D], bf16)
        d2 = small.tile([Bc, D], bf16)
        d3 = small.tile([Bc, D], bf16)
        o_sb = small.tile([Bc, D], f32)
        for chn in range(2):
            cs = slice(chn * 512, (chn + 1) * 512)
            gp, _ = gp_of[("h", chn)]
            nc.scalar.activation(out=h_sb[:, cs], in_=gp[:], func=AF.Tanh)
            nc.vector.tensor_tensor(out=d1[:, cs], in0=ss[:, cs],
                                    in1=h_sb[:, cs], op=ALU.subtract)
            nc.gpsimd.tensor_tensor(out=d2[:, cs], in0=d1[:, cs],
                                    in1=z_sb[:, cs], op=ALU.mult)
            nc.vector.tensor_tensor(out=d3[:, cs], in0=d2[:, cs],
                                    in1=h_sb[:, cs], op=ALU.add)
            nc.vector.tensor_tensor(out=o_sb[:, cs], in0=d3[:, cs],
                                    in1=v_un[:, cs], op=ALU.add)
            nc.sync.dma_start(o_d[:, cs], o_sb[:, cs])
        es.close()

      if loop_n == 1:
          body(0)
      else:
          with tc.For_i(0, loop_n, 1) as i:
              body(i)

    nc.compile()
    return nc


_NC_CACHE = {}


def _get_nc(loop_n=1):
    if loop_n not in _NC_CACHE:
        _NC_CACHE[loop_n] = _build(loop_n=loop_n)
    return _NC_CACHE[loop_n]


def _in_maps(inputs):
    def bf(a):
        return np.ascontiguousarray(np.asarray(a).astype(BF16_NP))

    def pack(w):
        # [T*128, D] -> [128, T, D] partition-major k-tile layout
        a = np.asarray(w).astype(BF16_NP)
        t = a.shape[0] // 128
        return np.ascontiguousarray(
            a.reshape(t, 128, a.shape[1]).transpose(1, 0, 2))

    watt_c = bf(np.asarray(inputs["w_att"], np.float32)[D:2 * D, 0]
                ).reshape(1, D)
    shared = {"w_att_c": watt_c}
    for g in "zrh":
        shared[f"w_{g}"] = pack(inputs[f"w_{g}"])
        shared[f"u_{g}"] = pack(inputs[f"u_{g}"])
        shared[f"b_{g}"] = bf(inputs[f"b_{g}"]).reshape(1, D)
    x_bf = bf(inputs["x"])
    s_bf = bf(inputs["state"])
    c_bf = bf(inputs["constants"])
    maps = []
    for c in range(N_CORES):
        lo, hi = c * Bc, (c + 1) * Bc
        m = dict(shared)
        m["x"] = np.ascontiguousarray(x_bf[lo:hi])
        m["state"] = np.ascontiguousarray(s_bf[lo:hi])
        m["constants"] = np.ascontiguousarray(c_bf[lo:hi])
        maps.append(m)
    return maps


def kernel(**inputs) -> np.ndarray:
    nc = _get_nc(loop_n=1)
    res = run_bass_kernel_spmd(nc, _in_maps(inputs),
                               core_ids=list(range(N_CORES)))
    return np.concatenate([res.results[c]["out"] for c in range(N_CORES)],
                          axis=0).astype(np.float32)


# revision 40
# speedup vs baseline: 2.1559x; 1.0618x over previous
"""nn_AttnDecoderCell — Trainium2 Bass kernel (8 NeuronCores, data-parallel).

kernel(**inputs) takes the FULL unsharded f32 inputs (x[512,1024],
state[512,1024], constants[512,256,1024], w_att[2048,1], b_att[1],
w_z/u_z/b_z, w_r/u_r/b_r, w_h/u_h/b_h) and returns the full s_t
[512, 1024] float32.

Sharding: batch dim split 64 rows per core; weights replicated. All inputs
are cast to bf16 on the host before upload — halves HBM traffic (the kernel
is DMA-bound) and runs the PE at 1 cycle/row instead of f32's 4.

Per-core program:
 - Attention: C streamed as [l(128p), d] bf16 tiles; energy = fused
   multiply+free-dim-reduce (scalar_tensor_tensor with accum_out) against a
   broadcast w_att row, split DVE/Pool (the state@w_att and b_att terms are
   softmax-shift-invariant); exp on ACT; v rows accumulated in PSUM with the
   exp column [128,1] stationary and C [128,512] moving, 4 batches landing
   on psum partitions {0,32,64,96} via col-group tile_position so one ACT
   copy stages 4 rows at once; staged rows PE-transposed into vT[128,8,64]
   (the layout the GRU needs) and transposed back once at the end for the
   [64,1024] row-layout v; sumexp via 2 ones-matmuls; normalization applied
   to vT once after the loop.
 - GRU: bf16 matmuls with batch on PSUM partitions; state.T/x.T/(r*state).T
   built via PE transposes; weights streamed from DRAM bf16 one whole matrix
   per DMA; bias added with a rank-1 ones matmul into the same PSUM group;
   sigmoid/tanh on ACT straight from PSUM; final combine on DVE/Pool.
"""

from contextlib import ExitStack

import numpy as np

import concourse.bacc as bacc
import concourse.bass as bass
import concourse.tile as tile
import concourse.mybir as mybir
from concourse.bass_utils import run_bass_kernel_spmd
from concourse.masks import make_identity

f32 = mybir.dt.float32
bf16 = mybir.dt.bfloat16
BF16_NP = mybir.dt.np(bf16)
AF = mybir.ActivationFunctionType
ALU = mybir.AluOpType

B, L, D, DIN = 512, 256, 1024, 1024
N_CORES = 8
Bc = B // N_CORES          # 64 batch rows per core
LT = L // 128              # 2 l-tiles
KW = D // 128              # 8 k-tiles for W matmuls
KU = (DIN + D) // 128      # 16 k-tiles for U matmuls
G = 4                      # batches per attention group (psum col-groups)
NG = Bc // G               # 16 groups


def _build(loop_n=1, c_bufs=3):
    nc = bacc.Bacc("TRN2", target_bir_lowering=False, debug=False,
                   num_devices=N_CORES)
    x_d = nc.dram_tensor("x", [Bc, DIN], bf16, kind="ExternalInput").ap()
    s_d = nc.dram_tensor("state", [Bc, D], bf16, kind="ExternalInput").ap()
    c_d = nc.dram_tensor("constants", [Bc, L, D], bf16,
                         kind="ExternalInput").ap()
    watt_d = nc.dram_tensor("w_att_c", [1, D], bf16, kind="ExternalInput").ap()
    w_g, u_g, b_g = {}, {}, {}
    for g in "zrh":
        # host-packed partition-major k-tile layouts (contiguous DMA blobs)
        w_g[g] = nc.dram_tensor(f"w_{g}", [128, KW, D], bf16,
                                kind="ExternalInput").ap()
        u_g[g] = nc.dram_tensor(f"u_{g}", [128, KU, D], bf16,
                                kind="ExternalInput").ap()
        b_g[g] = nc.dram_tensor(f"b_{g}", [1, D], bf16,
                                kind="ExternalInput").ap()
    o_d = nc.dram_tensor("out", [Bc, D], f32, kind="ExternalOutput").ap()

    with tile.TileContext(nc) as tc:
      def body(_i):
        es = ExitStack()
        small = es.enter_context(tc.tile_pool(name="small", bufs=1))
        cpool = es.enter_context(tc.tile_pool(name="cpool", bufs=c_bufs))
        scr = es.enter_context(tc.tile_pool(name="scr", bufs=2))
        st4 = es.enter_context(tc.tile_pool(name="st4", bufs=2))
        wpool = es.enter_context(tc.tile_pool(name="wpool", bufs=2))
        upool = es.enter_context(tc.tile_pool(name="upool", bufs=2))
        psV = es.enter_context(tc.tile_pool(name="psV", bufs=1, space="PSUM"))
        psT = es.enter_context(tc.tile_pool(name="psT", bufs=2, space="PSUM"))
        psG = es.enter_context(tc.tile_pool(name="psG", bufs=4, space="PSUM"))

        # ---------------- setup ----------------
        ident = small.tile([128, 128], bf16)
        make_identity(nc, ident[:])
        wc_rep = small.tile([128, D], bf16)
        nc.sync.dma_start(
            wc_rep[:],
            bass.AP(tensor=watt_d.tensor, offset=0, ap=[[0, 128], [1, D]]))
        ones_col = small.tile([128, 1], bf16)
        nc.vector.memset(ones_col[:], 1.0)
        ones_row = small.tile([1, Bc], bf16)
        nc.vector.memset(ones_row[:], 1.0)
        brow = {}
        for g in "zrh":
            brow[g] = small.tile([1, D], bf16, name=f"brow_{g}")
            nc.sync.dma_start(brow[g][:], b_g[g][:])

        # weight loads are issued AFTER the C stream (DMA drains in FIFO
        # order; C feeds the attention phase which runs first); contiguous
        # host-packed blobs
        def load_w(g):
            wt = wpool.tile([128, KW, D], bf16, tag="wt", name=f"w{g}")
            nc.sync.dma_start(wt[:], w_g[g][:])
            ut = upool.tile([128, KU, D], bf16, tag="ut", name=f"u{g}")
            nc.sync.dma_start(ut[:], u_g[g][:])
            return wt, ut

        xs = small.tile([Bc, DIN], bf16)
        nc.sync.dma_start(xs[:], x_d[:])
        ss = small.tile([Bc, D], bf16)
        nc.sync.dma_start(ss[:], s_d[:])

        def transpose_to(dst3, src2d):
            n = dst3.shape[1]
            for ch in range(n):
                tp = psT.tile([128, 8, 128], bf16, name="tp", tag="tp")
                nc.tensor.transpose(tp[:, 0, :Bc],
                                    src2d[:, ch * 128:(ch + 1) * 128],
                                    ident[:Bc, :Bc])
                nc.vector.tensor_copy(out=dst3[:, ch, :], in_=tp[:, 0, :Bc])

        sT = small.tile([128, KW, Bc], bf16)
        transpose_to(sT, ss)
        xT = small.tile([128, KW, Bc], bf16)
        transpose_to(xT, xs)

        # ---------------- attention ----------------
        eT = small.tile([128, LT * Bc], f32)
        expT = small.tile([128, LT * Bc], bf16)
        vT = small.tile([128, KW, Bc], bf16)

        vp = psV.tile([128, 2 * 512], f32)
        nc.vector.memset(vp[:], 0.0)

        for gi in range(NG):
            b0 = gi * G
            ct = cpool.tile([128, G, LT, D], bf16, tag="ct")
            nc.sync.dma_start(
                ct[:],
                c_d[b0:b0 + G].rearrange("b (t p) d -> p b t d", p=128))
            # energy: e[l, b] = sum_d C[l, d] * w_att[d]
            # 5/8 fused on DVE; 3/8 as Pool-mult + ACT Copy-accum reduce
            for bi in range(G):
                for lt in range(LT):
                    prod = scr.tile([128, D], bf16, tag=f"prod{bi}_{lt}",
                                    bufs=1)
                    col = lt * Bc + b0 + bi
                    if (bi * LT + lt) % 8 >= 5:
                        nc.gpsimd.tensor_tensor(
                            out=prod[:], in0=ct[:, bi, lt, :],
                            in1=wc_rep[:], op=ALU.mult)
                        nc.scalar.activation(
                            out=prod[:], in_=prod[:], func=AF.Copy,
                            accum_out=eT[:, col:col + 1])
                    else:
                        nc.vector.scalar_tensor_tensor(
                            out=prod[:], in0=ct[:, bi, lt, :], scalar=1.0,
                            in1=wc_rep[:], op0=ALU.bypass, op1=ALU.mult,
                            accum_out=eT[:, col:col + 1])
            for lt in range(LT):
                col = lt * Bc + b0
                nc.scalar.activation(out=expT[:, col:col + G],
                                     in_=eT[:, col:col + G], func=AF.Exp)
            # v rows into psum partitions {0,32,64,96} via col-groups
            for lt in range(LT):
                for chn in range(2):
                    for j in range(G):
                        nc.tensor.matmul(
                            vp[32 * j:32 * j + 1, chn * 512:(chn + 1) * 512],
                            expT[:, lt * Bc + b0 + j:lt * Bc + b0 + j + 1],
                            ct[:, j, lt, chn * 512:(chn + 1) * 512],
                            start=(lt == 0), stop=(lt == LT - 1),
                            skip_group_check=True, tile_position=(0, 32 * j))
            stage4 = st4.tile([128, D], bf16, tag="stage4")
            nc.scalar.copy(stage4[:], vp[:])
            # transpose the 4 staged rows into vT columns
            tp = psT.tile([128, 8, 128], bf16, name="tpv", tag="tp")
            for ch in range(KW):
                nc.tensor.transpose(tp[:, ch, :],
                                    stage4[:, ch * 128:(ch + 1) * 128],
                                    ident[:])
            nc.vector.tensor_copy(out=vT[:, :, b0:b0 + G],
                                  in_=tp[:, :, 0:97:32])

        # sumexp and normalization of vT
        s_ps = psT.tile([1, Bc], f32, name="s_ps", tag="tp")
        for lt in range(LT):
            nc.tensor.matmul(s_ps[:], ones_col[:],
                             expT[:, lt * Bc:(lt + 1) * Bc],
                             start=(lt == 0), stop=(lt == LT - 1),
                             skip_group_check=True)
        recip_row = small.tile([1, Bc], f32)
        nc.vector.reciprocal(recip_row[:], s_ps[:])
        recip_rep = small.tile([128, Bc], f32)
        nc.gpsimd.partition_broadcast(recip_rep[:], recip_row[:])
        for t in range(2):
            nc.vector.tensor_tensor(
                out=vT[:, 4 * t:4 * t + 4, :], in0=vT[:, 4 * t:4 * t + 4, :],
                in1=recip_rep[:, None, :].broadcast_to([128, 4, Bc]),
                op=ALU.mult)
        # transpose normalized vT back to row-layout v
        v_un = small.tile([Bc, D], bf16)
        for ch in range(KW):
            tb = psT.tile([128, 8, 128], bf16, name="tb", tag="tp")
            nc.tensor.transpose(tb[:Bc, 0, :], vT[:, ch, :], ident[:])
            nc.scalar.copy(v_un[:, ch * 128:(ch + 1) * 128], tb[:Bc, 0, :])

        # ---------------- GRU ----------------
        # Weight DMAs are stamped with tile_wait_until so the scheduler
        # cannot hoist them into the just-in-time C stream. uh/wh are
        # additionally slot-gated (3rd tile of their pools). Order matches
        # first use; wh last because h's W part has the shortest tail.
        with tc.tile_wait_until(0.088):
            wz, uz = load_w("z")
        with tc.tile_wait_until(0.105):
            wr, ur = load_w("r")
        with tc.tile_wait_until(0.118):
            uh = upool.tile([128, KU, D], bf16, tag="ut", name="uh")
            nc.sync.dma_start(uh[:], u_g["h"][:])
        with tc.tile_wait_until(0.140):
            wh = wpool.tile([128, KW, D], bf16, tag="wt", name="wh")
            nc.sync.dma_start(wh[:], w_g["h"][:])

        gp_of = {}

        def gate_mms(g, parts, wt=None, ut=None, lhsW3=None, stop=False):
            for chn in range(2):
                if (g, chn) not in gp_of:
                    gp_of[(g, chn)] = (
                        psG.tile([Bc, 512], f32, name=f"gp_{g}{chn}",
                                 tag="gp"), [True])
                gp, first = gp_of[(g, chn)]
                cs = slice(chn * 512, (chn + 1) * 512)

                def mm(lhs, rhs, last=False):
                    nc.tensor.matmul(gp[:], lhs, rhs, start=first[0],
                                     stop=stop and last,
                                     skip_group_check=True)
                    first[0] = False

                for p, part in enumerate(parts):
                    lastp = p == len(parts) - 1
                    if part == "w":
                        for k in range(KW):
                            mm(lhsW3[:, k, :], wt[:, k, cs],
                               last=lastp and k == KW - 1)
                    elif part == "ux":
                        for k in range(KW):
                            mm(xT[:, k, :], ut[:, k, cs],
                               last=lastp and k == KW - 1)
                    elif part == "uv":
                        for k in range(KW, KU):
                            mm(vT[:, k - KW, :], ut[:, k, cs],
                               last=lastp and k == KU - 1)
                    elif part == "bias":
                        mm(ones_row[:], brow[g][:, cs], last=lastp)

        def gate_act(g, out_sb, func):
            for chn in range(2):
                gp, _ = gp_of[(g, chn)]
                nc.scalar.activation(out=out_sb[:, chn * 512:(chn + 1) * 512],
                                     in_=gp[:], func=func)

        z_sb = small.tile([Bc, D], bf16)
        r_sb = small.tile([Bc, D], bf16)
        h_sb = small.tile([Bc, D], bf16)
        gate_mms("z", ["w", "ux", "uv", "bias"], wt=wz, ut=uz, lhsW3=sT,
                 stop=True)
        gate_act("z", z_sb, AF.Sigmoid)
        gate_mms("r", ["w", "ux", "uv", "bias"], wt=wr, ut=ur, lhsW3=sT,
                 stop=True)
        gate_act("r", r_sb, AF.Sigmoid)
        # h: Ux/Uv first (only need uh), W(rsT) last so the wh DMA and the
        # r -> rs -> rsT chain are both off the critical path as long as
        # possible
        gate_mms("h", ["ux", "uv", "bias"], ut=uh)
        rs_sb = small.tile([Bc, D], bf16)
        nc.vector.tensor_tensor(out=rs_sb[:], in0=ss[:], in1=r_sb[:],
                                op=ALU.mult)
        rsT = small.tile([128, KW, Bc], bf16)
        transpose_to(rsT, rs_sb)
        gate_mms("h", ["w"], wt=wh, lhsW3=rsT, stop=True)
        # per-half tail: tanh -> combine -> out DMA for each 512-column
        # half pipelines against the other half's matmuls
        d1 = small.tile([Bc, D], bf16)
        d2 = small.tile([Bc, D], bf16)
        d3 = small.tile([Bc, D], bf16)
        o_sb = small.tile([Bc, D], f32)
        for chn in range(2):
            cs = slice(chn * 512, (chn + 1) * 512)
            gp, _ = gp_of[("h", chn)]
            nc.scalar.activation(out=h_sb[:, cs], in_=gp[:], func=AF.Tanh)
            nc.vector.tensor_tensor(out=d1[:, cs], in0=ss[:, cs],
                                    in1=h_sb[:, cs], op=ALU.subtract)
            nc.gpsimd.tensor_tensor(out=d2[:, cs], in0=d1[:, cs],
                                    in1=z_sb[:, cs], op=ALU.mult)
            nc.vector.tensor_tensor(out=d3[:, cs], in0=d2[:, cs],
                                    in1=h_sb[:, cs], op=ALU.add)
            nc.vector.tensor_tensor(out=o_sb[:, cs], in0=d3[:, cs],
                                    in1=v_un[:, cs], op=ALU.add)
            nc.sync.dma_start(o_d[:, cs], o_sb[:, cs])
        es.close()

      if loop_n == 1:
          body(0)
      else:
          with tc.For_i(0, loop_n, 1) as i:
              body(i)

    nc.compile()
    return nc


_NC_CACHE = {}


def _get_nc(loop_n=1):
    if loop_n not in _NC_CACHE:
        _NC_CACHE[loop_n] = _build(loop_n=loop_n)
    return _NC_CACHE[loop_n]


def _in_maps(inputs):
    def bf(a):
        return np.ascontiguousarray(np.asarray(a).astype(BF16_NP))

    def pack(w):
        # [T*128, D] -> [128, T, D] partition-major k-tile layout
        a = np.asarray(w).astype(BF16_NP)
        t = a.shape[0] // 128
        return np.ascontiguousarray(
            a.reshape(t, 128, a.shape[1]).transpose(1, 0, 2))

    watt_c = bf(np.asarray(inputs["w_att"], np.float32)[D:2 * D, 0]
                ).reshape(1, D)
    shared = {"w_att_c": watt_c}
    for g in "zrh":
        shared[f"w_{g}"] = pack(inputs[f"w_{g}"])
        shared[f"u_{g}"] = pack(inputs[f"u_{g}"])
        shared[f"b_{g}"] = bf(inputs[f"b_{g}"]).reshape(1, D)
    x_bf = bf(inputs["x"])
    s_bf = bf(inputs["state"])
    c_bf = bf(inputs["constants"])
    maps = []
    for c in range(N_CORES):
        lo, hi = c * Bc, (c + 1) * Bc
        m = dict(shared)
        m["x"] = np.ascontiguousarray(x_bf[lo:hi])
        m["state"] = np.ascontiguousarray(s_bf[lo:hi])
        m["constants"] = np.ascontiguousarray(c_bf[lo:hi])
        maps.append(m)
    return maps


def kernel(**inputs) -> np.ndarray:
    nc = _get_nc(loop_n=1)
    res = run_bass_kernel_spmd(nc, _in_maps(inputs),
                               core_ids=list(range(N_CORES)))
    return np.concatenate([res.results[c]["out"] for c in range(N_CORES)],
                          axis=0).astype(np.float32)


# revision 41
# speedup vs baseline: 2.2313x; 1.0349x over previous
"""nn_AttnDecoderCell — Trainium2 Bass kernel (8 NeuronCores, data-parallel).

kernel(**inputs) takes the FULL unsharded f32 inputs (x[512,1024],
state[512,1024], constants[512,256,1024], w_att[2048,1], b_att[1],
w_z/u_z/b_z, w_r/u_r/b_r, w_h/u_h/b_h) and returns the full s_t
[512, 1024] float32.

Sharding: batch dim split 64 rows per core; weights replicated. All inputs
are cast to bf16 on the host before upload — halves HBM traffic (the kernel
is DMA-bound) and runs the PE at 1 cycle/row instead of f32's 4.

Per-core program:
 - Attention: C streamed as [l(128p), d] bf16 tiles; energy = fused
   multiply+free-dim-reduce (scalar_tensor_tensor with accum_out) against a
   broadcast w_att row, split DVE/Pool (the state@w_att and b_att terms are
   softmax-shift-invariant); exp on ACT; v rows accumulated in PSUM with the
   exp column [128,1] stationary and C [128,512] moving, 4 batches landing
   on psum partitions {0,32,64,96} via col-group tile_position so one ACT
   copy stages 4 rows at once; staged rows PE-transposed into vT[128,8,64]
   (the layout the GRU needs) and transposed back once at the end for the
   [64,1024] row-layout v; sumexp via 2 ones-matmuls; normalization applied
   to vT once after the loop.
 - GRU: bf16 matmuls with batch on PSUM partitions; state.T/x.T/(r*state).T
   built via PE transposes; weights streamed from DRAM bf16 one whole matrix
   per DMA; bias added with a rank-1 ones matmul into the same PSUM group;
   sigmoid/tanh on ACT straight from PSUM; final combine on DVE/Pool.
"""

from contextlib import ExitStack

import numpy as np

import concourse.bacc as bacc
import concourse.bass as bass
import concourse.tile as tile
import concourse.mybir as mybir
from concourse.bass_utils import run_bass_kernel_spmd
from concourse.masks import make_identity

f32 = mybir.dt.float32
bf16 = mybir.dt.bfloat16
BF16_NP = mybir.dt.np(bf16)
AF = mybir.ActivationFunctionType
ALU = mybir.AluOpType

B, L, D, DIN = 512, 256, 1024, 1024
N_CORES = 8
Bc = B // N_CORES          # 64 batch rows per core
LT = L // 128              # 2 l-tiles
KW = D // 128              # 8 k-tiles for W matmuls
KU = (DIN + D) // 128      # 16 k-tiles for U matmuls
G = 4                      # batches per attention group (psum col-groups)
NG = Bc // G               # 16 groups


def _build(loop_n=1, c_bufs=3):
    nc = bacc.Bacc("TRN2", target_bir_lowering=False, debug=False,
                   num_devices=N_CORES)
    x_d = nc.dram_tensor("x", [Bc, DIN], bf16, kind="ExternalInput").ap()
    s_d = nc.dram_tensor("state", [Bc, D], bf16, kind="ExternalInput").ap()
    c_d = nc.dram_tensor("constants", [Bc, L, D], bf16,
                         kind="ExternalInput").ap()
    watt_d = nc.dram_tensor("w_att_c", [1, D], bf16, kind="ExternalInput").ap()
    w_g, u_g, b_g = {}, {}, {}
    for g in "zrh":
        # host-packed partition-major k-tile layouts (contiguous DMA blobs)
        w_g[g] = nc.dram_tensor(f"w_{g}", [128, KW, D], bf16,
                                kind="ExternalInput").ap()
        u_g[g] = nc.dram_tensor(f"u_{g}", [128, KU, D], bf16,
                                kind="ExternalInput").ap()
        b_g[g] = nc.dram_tensor(f"b_{g}", [1, D], bf16,
                                kind="ExternalInput").ap()
    o_d = nc.dram_tensor("out", [Bc, D], f32, kind="ExternalOutput").ap()

    with tile.TileContext(nc) as tc:
      def body(_i):
        es = ExitStack()
        small = es.enter_context(tc.tile_pool(name="small", bufs=1))
        cpool = es.enter_context(tc.tile_pool(name="cpool", bufs=c_bufs))
        scr = es.enter_context(tc.tile_pool(name="scr", bufs=2))
        st4 = es.enter_context(tc.tile_pool(name="st4", bufs=2))
        wpool = es.enter_context(tc.tile_pool(name="wpool", bufs=2))
        upool = es.enter_context(tc.tile_pool(name="upool", bufs=2))
        psV = es.enter_context(tc.tile_pool(name="psV", bufs=1, space="PSUM"))
        psT = es.enter_context(tc.tile_pool(name="psT", bufs=2, space="PSUM"))
        psG = es.enter_context(tc.tile_pool(name="psG", bufs=4, space="PSUM"))

        # ---------------- setup ----------------
        ident = small.tile([128, 128], bf16)
        make_identity(nc, ident[:])
        wc_rep = small.tile([128, D], bf16)
        nc.sync.dma_start(
            wc_rep[:],
            bass.AP(tensor=watt_d.tensor, offset=0, ap=[[0, 128], [1, D]]))
        ones_col = small.tile([128, 1], bf16)
        nc.vector.memset(ones_col[:], 1.0)
        ones_row = small.tile([1, Bc], bf16)
        nc.vector.memset(ones_row[:], 1.0)
        brow = {}
        for g in "zrh":
            brow[g] = small.tile([1, D], bf16, name=f"brow_{g}")
            nc.sync.dma_start(brow[g][:], b_g[g][:])

        # weight loads are issued AFTER the C stream (DMA drains in FIFO
        # order; C feeds the attention phase which runs first); contiguous
        # host-packed blobs
        def load_w(g):
            wt = wpool.tile([128, KW, D], bf16, tag="wt", name=f"w{g}")
            nc.sync.dma_start(wt[:], w_g[g][:])
            ut = upool.tile([128, KU, D], bf16, tag="ut", name=f"u{g}")
            nc.sync.dma_start(ut[:], u_g[g][:])
            return wt, ut

        xs = small.tile([Bc, DIN], bf16)
        nc.sync.dma_start(xs[:], x_d[:])
        ss = small.tile([Bc, D], bf16)
        nc.sync.dma_start(ss[:], s_d[:])

        def transpose_to(dst3, src2d):
            n = dst3.shape[1]
            for ch in range(n):
                tp = psT.tile([128, 8, 128], bf16, name="tp", tag="tp")
                nc.tensor.transpose(tp[:, 0, :Bc],
                                    src2d[:, ch * 128:(ch + 1) * 128],
                                    ident[:Bc, :Bc])
                nc.vector.tensor_copy(out=dst3[:, ch, :], in_=tp[:, 0, :Bc])

        sT = small.tile([128, KW, Bc], bf16)
        transpose_to(sT, ss)
        xT = small.tile([128, KW, Bc], bf16)
        transpose_to(xT, xs)

        # ---------------- attention ----------------
        eT = small.tile([128, LT * Bc], f32)
        expT = small.tile([128, LT * Bc], bf16)
        vT = small.tile([128, KW, Bc], bf16)

        vp = psV.tile([128, 2 * 512], f32)
        nc.vector.memset(vp[:], 0.0)

        for gi in range(NG):
            b0 = gi * G
            ct = cpool.tile([128, G, LT, D], bf16, tag="ct")
            nc.sync.dma_start(
                ct[:],
                c_d[b0:b0 + G].rearrange("b (t p) d -> p b t d", p=128))
            # energy: e[l, b] = sum_d C[l, d] * w_att[d]
            # 5/8 fused on DVE; 3/8 as Pool-mult + ACT Copy-accum reduce
            for bi in range(G):
                for lt in range(LT):
                    prod = scr.tile([128, D], bf16, tag=f"prod{bi}_{lt}",
                                    bufs=1)
                    col = lt * Bc + b0 + bi
                    if (bi * LT + lt) % 8 >= 5:
                        nc.gpsimd.tensor_tensor(
                            out=prod[:], in0=ct[:, bi, lt, :],
                            in1=wc_rep[:], op=ALU.mult)
                        nc.scalar.activation(
                            out=prod[:], in_=prod[:], func=AF.Copy,
                            accum_out=eT[:, col:col + 1])
                    else:
                        nc.vector.scalar_tensor_tensor(
                            out=prod[:], in0=ct[:, bi, lt, :], scalar=1.0,
                            in1=wc_rep[:], op0=ALU.bypass, op1=ALU.mult,
                            accum_out=eT[:, col:col + 1])
            for lt in range(LT):
                col = lt * Bc + b0
                nc.scalar.activation(out=expT[:, col:col + G],
                                     in_=eT[:, col:col + G], func=AF.Exp)
            # v rows into psum partitions {0,32,64,96} via col-groups
            for lt in range(LT):
                for chn in range(2):
                    for j in range(G):
                        nc.tensor.matmul(
                            vp[32 * j:32 * j + 1, chn * 512:(chn + 1) * 512],
                            expT[:, lt * Bc + b0 + j:lt * Bc + b0 + j + 1],
                            ct[:, j, lt, chn * 512:(chn + 1) * 512],
                            start=(lt == 0), stop=(lt == LT - 1),
                            skip_group_check=True, tile_position=(0, 32 * j))
            stage4 = st4.tile([128, D], bf16, tag="stage4")
            nc.scalar.copy(stage4[:], vp[:])
            # transpose the 4 staged rows into vT columns
            tp = psT.tile([128, 8, 128], bf16, name="tpv", tag="tp")
            for ch in range(KW):
                nc.tensor.transpose(tp[:, ch, :],
                                    stage4[:, ch * 128:(ch + 1) * 128],
                                    ident[:])
            nc.vector.tensor_copy(out=vT[:, :, b0:b0 + G],
                                  in_=tp[:, :, 0:97:32])

        # sumexp and normalization of vT
        s_ps = psT.tile([1, Bc], f32, name="s_ps", tag="tp")
        for lt in range(LT):
            nc.tensor.matmul(s_ps[:], ones_col[:],
                             expT[:, lt * Bc:(lt + 1) * Bc],
                             start=(lt == 0), stop=(lt == LT - 1),
                             skip_group_check=True)
        recip_row = small.tile([1, Bc], f32)
        nc.vector.reciprocal(recip_row[:], s_ps[:])
        recip_rep = small.tile([128, Bc], f32)
        nc.gpsimd.partition_broadcast(recip_rep[:], recip_row[:])
        for t in range(2):
            nc.vector.tensor_tensor(
                out=vT[:, 4 * t:4 * t + 4, :], in0=vT[:, 4 * t:4 * t + 4, :],
                in1=recip_rep[:, None, :].broadcast_to([128, 4, Bc]),
                op=ALU.mult)
        # transpose normalized vT back to row-layout v
        v_un = small.tile([Bc, D], bf16)
        for ch in range(KW):
            tb = psT.tile([128, 8, 128], bf16, name="tb", tag="tp")
            nc.tensor.transpose(tb[:Bc, 0, :], vT[:, ch, :], ident[:])
            nc.scalar.copy(v_un[:, ch * 128:(ch + 1) * 128], tb[:Bc, 0, :])

        # ---------------- GRU ----------------
        # Weight DMAs are stamped with tile_wait_until so the scheduler
        # cannot hoist them into the just-in-time C stream. uh/wh are
        # additionally slot-gated (3rd tile of their pools). Order matches
        # first use; wh last because h's W part has the shortest tail.
        with tc.tile_wait_until(0.088):
            wz, uz = load_w("z")
        with tc.tile_wait_until(0.105):
            wr, ur = load_w("r")
        with tc.tile_wait_until(0.118):
            uh = upool.tile([128, KU, D], bf16, tag="ut", name="uh")
            nc.sync.dma_start(uh[:], u_g["h"][:])
        with tc.tile_wait_until(0.140):
            wh = wpool.tile([128, KW, D], bf16, tag="wt", name="wh")
            nc.sync.dma_start(wh[:], w_g["h"][:])

        gp_of = {}

        def gate_mms(g, parts, wt=None, ut=None, lhsW3=None, stop=False):
            for chn in range(2):
                if (g, chn) not in gp_of:
                    gp_of[(g, chn)] = (
                        psG.tile([Bc, 512], f32, name=f"gp_{g}{chn}",
                                 tag="gp"), [True])
                gp, first = gp_of[(g, chn)]
                cs = slice(chn * 512, (chn + 1) * 512)

                def mm(lhs, rhs, last=False):
                    nc.tensor.matmul(gp[:], lhs, rhs, start=first[0],
                                     stop=stop and last,
                                     skip_group_check=True)
                    first[0] = False

                for p, part in enumerate(parts):
                    lastp = p == len(parts) - 1
                    if part == "w":
                        for k in range(KW):
                            mm(lhsW3[:, k, :], wt[:, k, cs],
                               last=lastp and k == KW - 1)
                    elif part == "ux":
                        for k in range(KW):
                            mm(xT[:, k, :], ut[:, k, cs],
                               last=lastp and k == KW - 1)
                    elif part == "uv":
                        for k in range(KW, KU):
                            mm(vT[:, k - KW, :], ut[:, k, cs],
                               last=lastp and k == KU - 1)
                    elif part == "bias":
                        mm(ones_row[:], brow[g][:, cs], last=lastp)

        def gate_act(g, out_sb, func):
            for chn in range(2):
                gp, _ = gp_of[(g, chn)]
                nc.scalar.activation(out=out_sb[:, chn * 512:(chn + 1) * 512],
                                     in_=gp[:], func=func)

        z_sb = small.tile([Bc, D], bf16)
        r_sb = small.tile([Bc, D], bf16)
        h_sb = small.tile([Bc, D], bf16)
        gate_mms("z", ["w", "ux", "uv", "bias"], wt=wz, ut=uz, lhsW3=sT,
                 stop=True)
        gate_act("z", z_sb, AF.Sigmoid)
        gate_mms("r", ["w", "ux", "uv", "bias"], wt=wr, ut=ur, lhsW3=sT,
                 stop=True)
        gate_act("r", r_sb, AF.Sigmoid)
        # h: Ux/Uv first (only need uh), W(rsT) last so the wh DMA and the
        # r -> rs -> rsT chain are both off the critical path as long as
        # possible
        gate_mms("h", ["ux", "uv", "bias"], ut=uh)
        rs_sb = small.tile([Bc, D], bf16)
        nc.vector.tensor_tensor(out=rs_sb[:], in0=ss[:], in1=r_sb[:],
                                op=ALU.mult)
        rsT = small.tile([128, KW, Bc], bf16)
        transpose_to(rsT, rs_sb)
        gate_mms("h", ["w"], wt=wh, lhsW3=rsT, stop=True)
        gate_act("h", h_sb, AF.Tanh)

        d1 = small.tile([Bc, D], bf16)
        nc.vector.tensor_tensor(out=d1[:], in0=ss[:], in1=h_sb[:],
                                op=ALU.subtract)
        d2 = small.tile([Bc, D], bf16)
        nc.gpsimd.tensor_tensor(out=d2[:], in0=d1[:], in1=z_sb[:],
                                op=ALU.mult)
        d3 = small.tile([Bc, D], bf16)
        nc.vector.tensor_tensor(out=d3[:], in0=d2[:], in1=h_sb[:], op=ALU.add)
        o_sb = small.tile([Bc, D], f32)
        nc.vector.tensor_tensor(out=o_sb[:], in0=d3[:], in1=v_un[:],
                                op=ALU.add)
        nc.sync.dma_start(o_d[:], o_sb[:])
        es.close()

      if loop_n == 1:
          body(0)
      else:
          with tc.For_i(0, loop_n, 1) as i:
              body(i)

    nc.compile()
    return nc


_NC_CACHE = {}


def _get_nc(loop_n=1):
    if loop_n not in _NC_CACHE:
        _NC_CACHE[loop_n] = _build(loop_n=loop_n)
    return _NC_CACHE[loop_n]


def _in_maps(inputs):
    def bf(a):
        return np.ascontiguousarray(np.asarray(a).astype(BF16_NP))

    def pack(w):
        # [T*128, D] -> [128, T, D] partition-major k-tile layout
        a = np.asarray(w).astype(BF16_NP)
        t = a.shape[0] // 128
        return np.ascontiguousarray(
            a.reshape(t, 128, a.shape[1]).transpose(1, 0, 2))

    watt_c = bf(np.asarray(inputs["w_att"], np.float32)[D:2 * D, 0]
                ).reshape(1, D)
    shared = {"w_att_c": watt_c}
    for g in "zrh":
        shared[f"w_{g}"] = pack(inputs[f"w_{g}"])
        shared[f"u_{g}"] = pack(inputs[f"u_{g}"])
        shared[f"b_{g}"] = bf(inputs[f"b_{g}"]).reshape(1, D)
    x_bf = bf(inputs["x"])
    s_bf = bf(inputs["state"])
    c_bf = bf(inputs["constants"])
    maps = []
    for c in range(N_CORES):
        lo, hi = c * Bc, (c + 1) * Bc
        m = dict(shared)
        m["x"] = np.ascontiguousarray(x_bf[lo:hi])
        m["state"] = np.ascontiguousarray(s_bf[lo:hi])
        m["constants"] = np.ascontiguousarray(c_bf[lo:hi])
        maps.append(m)
    return maps


def kernel(**inputs) -> np.ndarray:
    nc = _get_nc(loop_n=1)
    res = run_bass_kernel_spmd(nc, _in_maps(inputs),
                               core_ids=list(range(N_CORES)))
    return np.concatenate([res.results[c]["out"] for c in range(N_CORES)],
                          axis=0).astype(np.float32)


# revision 44
# speedup vs baseline: 2.3017x; 1.0316x over previous
"""nn_AttnDecoderCell — Trainium2 Bass kernel (8 NeuronCores, data-parallel).

kernel(**inputs) takes the FULL unsharded f32 inputs (x[512,1024],
state[512,1024], constants[512,256,1024], w_att[2048,1], b_att[1],
w_z/u_z/b_z, w_r/u_r/b_r, w_h/u_h/b_h) and returns the full s_t
[512, 1024] float32.

Sharding: batch dim split 64 rows per core; weights replicated. All inputs
are cast to bf16 on the host before upload — halves HBM traffic (the kernel
is DMA-bound) and runs the PE at 1 cycle/row instead of f32's 4.

Per-core program:
 - Attention: C streamed as [l(128p), d] bf16 tiles; energy = fused
   multiply+free-dim-reduce (scalar_tensor_tensor with accum_out) against a
   broadcast w_att row, split DVE/Pool (the state@w_att and b_att terms are
   softmax-shift-invariant); exp on ACT; v rows accumulated in PSUM with the
   exp column [128,1] stationary and C [128,512] moving, 4 batches landing
   on psum partitions {0,32,64,96} via col-group tile_position so one ACT
   copy stages 4 rows at once; staged rows PE-transposed into vT[128,8,64]
   (the layout the GRU needs) and transposed back once at the end for the
   [64,1024] row-layout v; sumexp via 2 ones-matmuls; normalization applied
   to vT once after the loop.
 - GRU: bf16 matmuls with batch on PSUM partitions; state.T/x.T/(r*state).T
   built via PE transposes; weights streamed from DRAM bf16 one whole matrix
   per DMA; bias added with a rank-1 ones matmul into the same PSUM group;
   sigmoid/tanh on ACT straight from PSUM; final combine on DVE/Pool.
"""

from contextlib import ExitStack

import numpy as np

import concourse.bacc as bacc
import concourse.bass as bass
import concourse.tile as tile
import concourse.mybir as mybir
from concourse.bass_utils import run_bass_kernel_spmd
from concourse.masks import make_identity

f32 = mybir.dt.float32
bf16 = mybir.dt.bfloat16
BF16_NP = mybir.dt.np(bf16)
AF = mybir.ActivationFunctionType
ALU = mybir.AluOpType

B, L, D, DIN = 512, 256, 1024, 1024
N_CORES = 8
Bc = B // N_CORES          # 64 batch rows per core
LT = L // 128              # 2 l-tiles
KW = D // 128              # 8 k-tiles for W matmuls
KU = (DIN + D) // 128      # 16 k-tiles for U matmuls
G = 4                      # batches per attention group (psum col-groups)
NG = Bc // G               # 16 groups


def _build(loop_n=1, c_bufs=3):
    nc = bacc.Bacc("TRN2", target_bir_lowering=False, debug=False,
                   num_devices=N_CORES)
    x_d = nc.dram_tensor("x", [Bc, DIN], bf16, kind="ExternalInput").ap()
    s_d = nc.dram_tensor("state", [Bc, D], bf16, kind="ExternalInput").ap()
    c_d = nc.dram_tensor("constants", [Bc, L, D], bf16,
                         kind="ExternalInput").ap()
    watt_d = nc.dram_tensor("w_att_c", [1, D], bf16, kind="ExternalInput").ap()
    w_g, u_g, b_g = {}, {}, {}
    for g in "zrh":
        # host-packed partition-major k-tile layouts (contiguous DMA blobs)
        w_g[g] = nc.dram_tensor(f"w_{g}", [128, KW, D], bf16,
                                kind="ExternalInput").ap()
        u_g[g] = nc.dram_tensor(f"u_{g}", [128, KU, D], bf16,
                                kind="ExternalInput").ap()
        b_g[g] = nc.dram_tensor(f"b_{g}", [1, D], bf16,
                                kind="ExternalInput").ap()
    o_d = nc.dram_tensor("out", [Bc, D], f32, kind="ExternalOutput").ap()

    with tile.TileContext(nc) as tc:
      def body(_i):
        es = ExitStack()
        small = es.enter_context(tc.tile_pool(name="small", bufs=1))
        cpool = es.enter_context(tc.tile_pool(name="cpool", bufs=c_bufs))
        scr = es.enter_context(tc.tile_pool(name="scr", bufs=2))
        st4 = es.enter_context(tc.tile_pool(name="st4", bufs=2))
        wpool = es.enter_context(tc.tile_pool(name="wpool", bufs=2))
        upool = es.enter_context(tc.tile_pool(name="upool", bufs=2))
        psV = es.enter_context(tc.tile_pool(name="psV", bufs=1, space="PSUM"))
        psT = es.enter_context(tc.tile_pool(name="psT", bufs=2, space="PSUM"))
        psG = es.enter_context(tc.tile_pool(name="psG", bufs=4, space="PSUM"))

        # ---------------- setup ----------------
        ident = small.tile([128, 128], bf16)
        make_identity(nc, ident[:])
        wc_rep = small.tile([128, D], bf16)
        nc.sync.dma_start(
            wc_rep[:],
            bass.AP(tensor=watt_d.tensor, offset=0, ap=[[0, 128], [1, D]]))
        ones_col = small.tile([128, 1], bf16)
        nc.vector.memset(ones_col[:], 1.0)
        ones_row = small.tile([1, Bc], bf16)
        nc.vector.memset(ones_row[:], 1.0)
        brow = {}
        for g in "zrh":
            brow[g] = small.tile([1, D], bf16, name=f"brow_{g}")
            nc.sync.dma_start(brow[g][:], b_g[g][:])

        # weight loads are issued AFTER the C stream (DMA drains in FIFO
        # order; C feeds the attention phase which runs first); contiguous
        # host-packed blobs
        def load_w(g):
            wt = wpool.tile([128, KW, D], bf16, tag="wt", name=f"w{g}")
            nc.sync.dma_start(wt[:], w_g[g][:])
            ut = upool.tile([128, KU, D], bf16, tag="ut", name=f"u{g}")
            nc.sync.dma_start(ut[:], u_g[g][:])
            return wt, ut

        xs = small.tile([Bc, DIN], bf16)
        nc.sync.dma_start(xs[:], x_d[:])
        ss = small.tile([Bc, D], bf16)
        nc.sync.dma_start(ss[:], s_d[:])

        def transpose_to(dst3, src2d):
            n = dst3.shape[1]
            for ch in range(n):
                tp = psT.tile([128, 8, 128], bf16, name="tp", tag="tp")
                nc.tensor.transpose(tp[:, 0, :Bc],
                                    src2d[:, ch * 128:(ch + 1) * 128],
                                    ident[:Bc, :Bc])
                nc.vector.tensor_copy(out=dst3[:, ch, :], in_=tp[:, 0, :Bc])

        sT = small.tile([128, KW, Bc], bf16)
        transpose_to(sT, ss)
        xT = small.tile([128, KW, Bc], bf16)
        transpose_to(xT, xs)

        # ---------------- attention ----------------
        eT = small.tile([128, LT * Bc], f32)
        expT = small.tile([128, LT * Bc], bf16)
        vT = small.tile([128, KW, Bc], bf16)

        vp = psV.tile([128, 2 * 512], f32)
        nc.vector.memset(vp[:], 0.0)

        wz = uz = wr = ur = None
        for gi in range(NG):
            if gi == 7:
                # z/r weights enter the DMA stream here: on HW the stream
                # runs ~2x the model's rate, so displacing a couple of C
                # tiles is cheap and the scheduler can fill attention-phase
                # PE idle with the GRU W/Ux matmuls
                wz, uz = load_w("z")
                wr, ur = load_w("r")
            b0 = gi * G
            ct = cpool.tile([128, G, LT, D], bf16, tag="ct")
            nc.sync.dma_start(
                ct[:],
                c_d[b0:b0 + G].rearrange("b (t p) d -> p b t d", p=128))
            # energy: e[l, b] = sum_d C[l, d] * w_att[d]
            # 5/8 fused on DVE; 3/8 as Pool-mult + ACT Copy-accum reduce
            for bi in range(G):
                for lt in range(LT):
                    prod = scr.tile([128, D], bf16, tag=f"prod{bi}_{lt}",
                                    bufs=1)
                    col = lt * Bc + b0 + bi
                    if (bi * LT + lt) % 8 >= 5:
                        nc.gpsimd.tensor_tensor(
                            out=prod[:], in0=ct[:, bi, lt, :],
                            in1=wc_rep[:], op=ALU.mult)
                        nc.scalar.activation(
                            out=prod[:], in_=prod[:], func=AF.Copy,
                            accum_out=eT[:, col:col + 1])
                    else:
                        nc.vector.scalar_tensor_tensor(
                            out=prod[:], in0=ct[:, bi, lt, :], scalar=1.0,
                            in1=wc_rep[:], op0=ALU.bypass, op1=ALU.mult,
                            accum_out=eT[:, col:col + 1])
            for lt in range(LT):
                col = lt * Bc + b0
                nc.scalar.activation(out=expT[:, col:col + G],
                                     in_=eT[:, col:col + G], func=AF.Exp)
            # v rows into psum partitions {0,32,64,96} via col-groups
            for lt in range(LT):
                for chn in range(2):
                    for j in range(G):
                        nc.tensor.matmul(
                            vp[32 * j:32 * j + 1, chn * 512:(chn + 1) * 512],
                            expT[:, lt * Bc + b0 + j:lt * Bc + b0 + j + 1],
                            ct[:, j, lt, chn * 512:(chn + 1) * 512],
                            start=(lt == 0), stop=(lt == LT - 1),
                            skip_group_check=True, tile_position=(0, 32 * j))
            stage4 = st4.tile([128, D], bf16, tag="stage4")
            nc.scalar.copy(stage4[:], vp[:])
            # transpose the 4 staged rows into vT columns
            tp = psT.tile([128, 8, 128], bf16, name="tpv", tag="tp")
            for ch in range(KW):
                nc.tensor.transpose(tp[:, ch, :],
                                    stage4[:, ch * 128:(ch + 1) * 128],
                                    ident[:])
            nc.vector.tensor_copy(out=vT[:, :, b0:b0 + G],
                                  in_=tp[:, :, 0:97:32])

        # sumexp and normalization of vT
        s_ps = psT.tile([1, Bc], f32, name="s_ps", tag="tp")
        for lt in range(LT):
            nc.tensor.matmul(s_ps[:], ones_col[:],
                             expT[:, lt * Bc:(lt + 1) * Bc],
                             start=(lt == 0), stop=(lt == LT - 1),
                             skip_group_check=True)
        recip_row = small.tile([1, Bc], f32)
        nc.vector.reciprocal(recip_row[:], s_ps[:])
        recip_rep = small.tile([128, Bc], f32)
        nc.gpsimd.partition_broadcast(recip_rep[:], recip_row[:])
        for t in range(2):
            nc.vector.tensor_tensor(
                out=vT[:, 4 * t:4 * t + 4, :], in0=vT[:, 4 * t:4 * t + 4, :],
                in1=recip_rep[:, None, :].broadcast_to([128, 4, Bc]),
                op=ALU.mult)
        # transpose normalized vT back to row-layout v
        v_un = small.tile([Bc, D], bf16)
        for ch in range(KW):
            tb = psT.tile([128, 8, 128], bf16, name="tb", tag="tp")
            nc.tensor.transpose(tb[:Bc, 0, :], vT[:, ch, :], ident[:])
            nc.scalar.copy(v_un[:, ch * 128:(ch + 1) * 128], tb[:Bc, 0, :])

        # ---------------- GRU ----------------
        # uh/wh are slot-gated (3rd tile of their pools — their DMAs wait
        # for z's weight reads to finish). wh last because h's W part has
        # the shortest tail.
        uh = upool.tile([128, KU, D], bf16, tag="ut", name="uh")
        nc.sync.dma_start(uh[:], u_g["h"][:])
        wh = wpool.tile([128, KW, D], bf16, tag="wt", name="wh")
        nc.sync.dma_start(wh[:], w_g["h"][:])

        gp_of = {}

        def gate_mms(g, parts, wt=None, ut=None, lhsW3=None, stop=False):
            for chn in range(2):
                if (g, chn) not in gp_of:
                    gp_of[(g, chn)] = (
                        psG.tile([Bc, 512], f32, name=f"gp_{g}{chn}",
                                 tag="gp"), [True])
                gp, first = gp_of[(g, chn)]
                cs = slice(chn * 512, (chn + 1) * 512)

                def mm(lhs, rhs, last=False):
                    nc.tensor.matmul(gp[:], lhs, rhs, start=first[0],
                                     stop=stop and last,
                                     skip_group_check=True)
                    first[0] = False

                for p, part in enumerate(parts):
                    lastp = p == len(parts) - 1
                    if part == "w":
                        for k in range(KW):
                            mm(lhsW3[:, k, :], wt[:, k, cs],
                               last=lastp and k == KW - 1)
                    elif part == "ux":
                        for k in range(KW):
                            mm(xT[:, k, :], ut[:, k, cs],
                               last=lastp and k == KW - 1)
                    elif part == "uv":
                        for k in range(KW, KU):
                            mm(vT[:, k - KW, :], ut[:, k, cs],
                               last=lastp and k == KU - 1)
                    elif part == "bias":
                        mm(ones_row[:], brow[g][:, cs], last=lastp)

        def gate_act(g, out_sb, func):
            for chn in range(2):
                gp, _ = gp_of[(g, chn)]
                nc.scalar.activation(out=out_sb[:, chn * 512:(chn + 1) * 512],
                                     in_=gp[:], func=func)

        z_sb = small.tile([Bc, D], bf16)
        r_sb = small.tile([Bc, D], bf16)
        h_sb = small.tile([Bc, D], bf16)
        gate_mms("z", ["w", "ux", "uv", "bias"], wt=wz, ut=uz, lhsW3=sT,
                 stop=True)
        gate_act("z", z_sb, AF.Sigmoid)
        gate_mms("r", ["w", "ux", "uv", "bias"], wt=wr, ut=ur, lhsW3=sT,
                 stop=True)
        gate_act("r", r_sb, AF.Sigmoid)
        # h: Ux/Uv first (only need uh), W(rsT) last so the wh DMA and the
        # r -> rs -> rsT chain are both off the critical path as long as
        # possible
        gate_mms("h", ["ux", "uv", "bias"], ut=uh)
        rs_sb = small.tile([Bc, D], bf16)
        nc.vector.tensor_tensor(out=rs_sb[:], in0=ss[:], in1=r_sb[:],
                                op=ALU.mult)
        rsT = small.tile([128, KW, Bc], bf16)
        transpose_to(rsT, rs_sb)
        gate_mms("h", ["w"], wt=wh, lhsW3=rsT, stop=True)
        gate_act("h", h_sb, AF.Tanh)

        d1 = small.tile([Bc, D], bf16)
        nc.vector.tensor_tensor(out=d1[:], in0=ss[:], in1=h_sb[:],
                                op=ALU.subtract)
        d2 = small.tile([Bc, D], bf16)
        nc.gpsimd.tensor_tensor(out=d2[:], in0=d1[:], in1=z_sb[:],
                                op=ALU.mult)
        d3 = small.tile([Bc, D], bf16)
        nc.vector.tensor_tensor(out=d3[:], in0=d2[:], in1=h_sb[:], op=ALU.add)
        o_sb = small.tile([Bc, D], f32)
        nc.vector.tensor_tensor(out=o_sb[:], in0=d3[:], in1=v_un[:],
                                op=ALU.add)
        nc.sync.dma_start(o_d[:], o_sb[:])
        es.close()

      if loop_n == 1:
          body(0)
      else:
          with tc.For_i(0, loop_n, 1) as i:
              body(i)

    nc.compile()
    return nc


_NC_CACHE = {}


def _get_nc(loop_n=1):
    if loop_n not in _NC_CACHE:
        _NC_CACHE[loop_n] = _build(loop_n=loop_n)
    return _NC_CACHE[loop_n]


def _in_maps(inputs):
    def bf(a):
        return np.ascontiguousarray(np.asarray(a).astype(BF16_NP))

    def pack(w):
        # [T*128, D] -> [128, T, D] partition-major k-tile layout
        a = np.asarray(w).astype(BF16_NP)
        t = a.shape[0] // 128
        return np.ascontiguousarray(
            a.reshape(t, 128, a.shape[1]).transpose(1, 0, 2))

    watt_c = bf(np.asarray(inputs["w_att"], np.float32)[D:2 * D, 0]
                ).reshape(1, D)
    shared = {"w_att_c": watt_c}
    for g in "zrh":
        shared[f"w_{g}"] = pack(inputs[f"w_{g}"])
        shared[f"u_{g}"] = pack(inputs[f"u_{g}"])
        shared[f"b_{g}"] = bf(inputs[f"b_{g}"]).reshape(1, D)
    x_bf = bf(inputs["x"])
    s_bf = bf(inputs["state"])
    c_bf = bf(inputs["constants"])
    maps = []
    for c in range(N_CORES):
        lo, hi = c * Bc, (c + 1) * Bc
        m = dict(shared)
        m["x"] = np.ascontiguousarray(x_bf[lo:hi])
        m["state"] = np.ascontiguousarray(s_bf[lo:hi])
        m["constants"] = np.ascontiguousarray(c_bf[lo:hi])
        maps.append(m)
    return maps


def kernel(**inputs) -> np.ndarray:
    nc = _get_nc(loop_n=1)
    res = run_bass_kernel_spmd(nc, _in_maps(inputs),
                               core_ids=list(range(N_CORES)))
    return np.concatenate([res.results[c]["out"] for c in range(N_CORES)],
                          axis=0).astype(np.float32)
